# revision 5
# baseline (speedup 1.0000x reference)
"""Multi-head causal attention (B=4, T=2048, C=1024, H=16, D=64) on 8 TRN2
NeuronCores via Bass/Tile: compensated fp16 hi/lo 3-pass matmuls.

Sharding (unchanged from v1): core id = 2*batch + g; g selects 8 of 16 heads.
Each core computes attention for its 8 heads over its batch element and
ReduceScatters the out-projection partial with its pair sibling.

v2 numerics: every GEMM except P@V runs as a compensated fp16 pair
(X ~ Xh + Xl, W ~ Wh + Wl; C = XhWh + XhWl + XlWh), which streams at
1 cycle/row on the PE (vs 4 for fp32) with ~fp32-grade accuracy
(HW-measured 2.6e-6 absmax for K=1024 vs fp32's 7.9e-7).
x and the weight matrices are pre-split on the host; Q/K and the
normalized attention output are split on-device with one fused DVE op
per half. P@V keeps fp32 P (splitting 16.8M P elements would swamp DVE).
"""
import sys

for _p in ("/opt/trn_rl_repo", "/root/.axon_site/_ro/trn_rl_repo"):
    if _p not in sys.path:
        sys.path.append(_p)

import numpy as np
import concourse.bass as bass
import concourse.tile as tile
from concourse import bacc, mybir
from concourse.bass_utils import run_bass_kernel_spmd

F32 = mybir.dt.float32
F16 = mybir.dt.float16
ALU = mybir.AluOpType

B, T, C = 4, 2048, 1024
H, D = 16, 64
NCORES = 8
HL = 8            # local heads per core
CL = HL * D       # 512 local channels
TCH = 512         # t-chunk (query tile)
NJ = T // TCH     # 4 chunks
KT = C // 128     # 8 contraction tiles for projections
KO = CL // 128    # 4 contraction tiles for out-proj
NEG = -1e10
SCL = float(D) ** -0.5


def build(collective=True, reps=1, ps_cfg=(1, 3, 2, 1), pp_bufs=6,
          stage="full", samerow=False):
    # stage: 'proj' | 'scores' | 'pv' | 'full' — truncated builds for
    # stage-wise HW timing (measure.py only; grading path is 'full').
    # samerow: force both score halves onto PE rows 0-63 (serializes the
    # head pair) to A/B-test dual row-group concurrency.
    nc = bacc.Bacc("TRN2", target_bir_lowering=False, debug=False,
                   num_devices=NCORES)
    x_h = nc.dram_tensor("x_h", [C, T], F16, kind="ExternalInput").ap()
    x_l = nc.dram_tensor("x_l", [C, T], F16, kind="ExternalInput").ap()
    w_h = nc.dram_tensor("w_h", [C, 3 * CL], F16, kind="ExternalInput").ap()
    w_l = nc.dram_tensor("w_l", [C, 3 * CL], F16, kind="ExternalInput").ap()
    wo_h = nc.dram_tensor("wo_h", [CL, C], F16, kind="ExternalInput").ap()
    wo_l = nc.dram_tensor("wo_l", [CL, C], F16, kind="ExternalInput").ap()
    b_eff = nc.dram_tensor("b_eff", [C], F32, kind="ExternalInput").ap()
    out_half = nc.dram_tensor("out_half", [CL, T], F32, kind="ExternalOutput").ap()

    with tile.TileContext(nc) as tc:
        with (
            tc.tile_pool(name="consts", bufs=1) as consts,
            tc.tile_pool(name="weights", bufs=1) as weights,
            tc.tile_pool(name="kv", bufs=1) as kv,
            tc.tile_pool(name="xin", bufs=1) as xin,
            tc.tile_pool(name="qp", bufs=1) as qp,
            tc.tile_pool(name="pp", bufs=pp_bufs) as pp,
            tc.tile_pool(name="att", bufs=1) as att,
            tc.tile_pool(name="sm", bufs=2) as sm,
            tc.tile_pool(name="outp", bufs=2) as outp,
            tc.tile_pool(name="ps_proj", bufs=ps_cfg[0], space="PSUM") as ps_proj,
            tc.tile_pool(name="ps_s", bufs=ps_cfg[1], space="PSUM") as ps_s,
            tc.tile_pool(name="ps_pv", bufs=ps_cfg[2], space="PSUM") as ps_pv,
            tc.tile_pool(name="ps_o", bufs=ps_cfg[3], space="PSUM") as ps_o,
            tc.tile_pool(name="ps_bc", bufs=1, space="PSUM") as ps_bc,
            tc.tile_pool(name="dram", bufs=2, space="DRAM") as dram,
        ):
            # ---- constants ----
            mask = consts.tile([128, 128], F32)
            nc.vector.memset(mask[:], 0.0)
            # keep 0 where f >= p (k <= q), else NEG
            nc.gpsimd.affine_select(
                out=mask[:], in_=mask[:], compare_op=mybir.AluOpType.is_ge,
                fill=NEG, base=0, pattern=[[1, 128]], channel_multiplier=-1,
            )
            ones_r = consts.tile([1, 64], F32)
            nc.vector.memset(ones_r[:], 1.0)
            b_sb = consts.tile([128, KT], F32)
            nc.sync.dma_start(b_sb[:], b_eff.rearrange("(mo p) -> p mo", p=128))

            # ---- weights (fp16 hi/lo pairs) ----
            w_th = weights.tile([128, KT, 3 * CL], F16)
            w_tl = weights.tile([128, KT, 3 * CL], F16)
            nc.sync.dma_start(w_th[:], w_h.rearrange("(kt p) n -> p kt n", p=128))
            nc.sync.dma_start(w_tl[:], w_l.rearrange("(kt p) n -> p kt n", p=128))
            wo_th = weights.tile([128, KO, C], F16)
            wo_tl = weights.tile([128, KO, C], F16)
            nc.sync.dma_start(wo_th[:], wo_h.rearrange("(ko p) n -> p ko n", p=128))
            nc.sync.dma_start(wo_tl[:], wo_l.rearrange("(ko p) n -> p ko n", p=128))

            # ---- persistent K^T pair and (ones-augmented) V (fp32) ----
            kt_h = kv.tile([128, KO, T], F16)
            kt_l = kv.tile([128, KO, T], F16)
            v_t = kv.tile([128, T // 128, HL * 65], F32)  # V rows = t, 65 cols/head
            v_aug = v_t.rearrange("p tt (h e) -> p tt h e", e=65)
            nc.vector.memset(v_aug[:, :, :, 64:65], 1.0)

            x_rh = x_h.rearrange("(kt p) t -> p kt t", p=128)
            x_rl = x_l.rearrange("(kt p) t -> p kt t", p=128)

            for _rep in range(reps):
              for j in range(NJ):
                ts = slice(j * TCH, (j + 1) * TCH)
                # ---- load x^T chunk (hi/lo) ----
                xc_h = xin.tile([128, KT, TCH], F16, tag="xh")
                xc_l = xin.tile([128, KT, TCH], F16, tag="xl")
                nc.sync.dma_start(xc_h[:], x_rh[:, :, ts])
                nc.sync.dma_start(xc_l[:], x_rl[:, :, ts])

                # ---- projections for this chunk (3-pass fp16 pairs) ----
                qt_h = qp.tile([128, KO, TCH], F16, tag="qh")
                qt_l = qp.tile([128, KO, TCH], F16, tag="ql")
                for m in range(KO):
                    psq = ps_proj.tile([128, TCH], F32, tag="proj")
                    ws = slice(128 * m, 128 * (m + 1))
                    for k in range(KT):
                        first, last = (k == 0), (k == KT - 1)
                        nc.tensor.matmul(psq[:], w_th[:, k, ws], xc_h[:, k, :],
                                         start=first, stop=False)
                        nc.tensor.matmul(psq[:], w_th[:, k, ws], xc_l[:, k, :],
                                         start=False, stop=False)
                        nc.tensor.matmul(psq[:], w_tl[:, k, ws], xc_h[:, k, :],
                                         start=False, stop=last)
                    nc.vector.tensor_scalar_mul(qt_h[:, m, :], psq[:], SCL)
                    nc.vector.scalar_tensor_tensor(
                        qt_l[:, m, :], psq[:], SCL, qt_h[:, m, :],
                        ALU.mult, ALU.subtract)
                for m in range(KO):
                    psk = ps_proj.tile([128, TCH], F32, tag="proj")
                    ws = slice(CL + 128 * m, CL + 128 * (m + 1))
                    for k in range(KT):
                        first, last = (k == 0), (k == KT - 1)
                        nc.tensor.matmul(psk[:], w_th[:, k, ws], xc_h[:, k, :],
                                         start=first, stop=False)
                        nc.tensor.matmul(psk[:], w_th[:, k, ws], xc_l[:, k, :],
                                         start=False, stop=False)
                        nc.tensor.matmul(psk[:], w_tl[:, k, ws], xc_h[:, k, :],
                                         start=False, stop=last)
                    nc.vector.tensor_copy(kt_h[:, m, ts], psk[:])
                    nc.vector.scalar_tensor_tensor(
                        kt_l[:, m, ts], psk[:], 1.0, kt_h[:, m, ts],
                        ALU.mult, ALU.subtract)
                for ttl in range(TCH // 128):
                    tt = j * (TCH // 128) + ttl
                    psv = ps_proj.tile([128, CL], F32, tag="proj")
                    xs = slice(128 * ttl, 128 * (ttl + 1))
                    ws = slice(2 * CL, 3 * CL)
                    for k in range(KT):
                        first, last = (k == 0), (k == KT - 1)
                        nc.tensor.matmul(psv[:], xc_h[:, k, xs], w_th[:, k, ws],
                                         start=first, stop=False)
                        nc.tensor.matmul(psv[:], xc_h[:, k, xs], w_tl[:, k, ws],
                                         start=False, stop=False)
                        nc.tensor.matmul(psv[:], xc_l[:, k, xs], w_th[:, k, ws],
                                         start=False, stop=last)
                    nc.vector.tensor_copy(
                        v_aug[:, tt, :, 0:64],
                        psv.rearrange("p (h d) -> p h d", h=HL))

                if stage == "proj":
                    continue
                # ---- attention for this chunk ----
                # two heads of a pair interleaved: their K=64 score matmuls
                # sit in different PE row groups (base partitions 0 / 64) and
                # run concurrently when adjacent in the instruction stream.
                at_c = att.tile([128, KO, TCH], F32, tag="at")
                at_h = att.tile([128, KO, TCH], F16, tag="ath")
                at_l = att.tile([128, KO, TCH], F16, tag="atl")
                for m in range(KO):
                    ha, hb = 2 * m, 2 * m + 1
                    pva = ps_pv.tile([65, TCH], F32, tag="pv")
                    pvb = ps_pv.tile([65, TCH], F32, tag="pv")
                    nkb = 4 * (j + 1)
                    for kb in range(nkb):
                        r = kb - 4 * j
                        off = 128 * max(r, 0)
                        ks = slice(128 * kb, 128 * (kb + 1))
                        spa = ps_s.tile([128, TCH], F32, tag="s")
                        spb = ps_s.tile([128, TCH], F32, tag="s")
                        nc.tensor.matmul(
                            spa[:, off:], kt_h[0:64, m, ks], qt_h[0:64, m, off:],
                            start=True, stop=False)
                        bs = slice(0, 64) if samerow else slice(64, 128)
                        nc.tensor.matmul(
                            spb[:, off:], kt_h[bs, m, ks], qt_h[bs, m, off:],
                            start=True, stop=False)
                        nc.tensor.matmul(
                            spa[:, off:], kt_h[0:64, m, ks], qt_l[0:64, m, off:],
                            start=False, stop=False)
                        nc.tensor.matmul(
                            spb[:, off:], kt_h[bs, m, ks], qt_l[bs, m, off:],
                            start=False, stop=False)
                        nc.tensor.matmul(
                            spa[:, off:], kt_l[0:64, m, ks], qt_h[0:64, m, off:],
                            start=False, stop=True)
                        nc.tensor.matmul(
                            spb[:, off:], kt_l[bs, m, ks], qt_h[bs, m, off:],
                            start=False, stop=True)
                        if r >= 0:
                            nc.vector.tensor_add(
                                spa[:, off:off + 128], spa[:, off:off + 128], mask[:])
                            nc.vector.tensor_add(
                                spb[:, off:off + 128], spb[:, off:off + 128], mask[:])
                        pa = pp.tile([128, TCH], F32, tag="p")
                        pb = pp.tile([128, TCH], F32, tag="p")
                        nc.scalar.activation(
                            pa[:, off:], spa[:, off:], mybir.ActivationFunctionType.Exp)
                        nc.scalar.activation(
                            pb[:, off:], spb[:, off:], mybir.ActivationFunctionType.Exp)
                        if stage in ("pv", "full"):
                            nc.tensor.matmul(
                                pva[:, off:], v_t[:, kb, 65 * ha:65 * ha + 65],
                                pa[:, off:], start=(kb == 0), stop=(kb == nkb - 1))
                            nc.tensor.matmul(
                                pvb[:, off:], v_t[:, kb, 65 * hb:65 * hb + 65],
                                pb[:, off:], start=(kb == 0), stop=(kb == nkb - 1))
                    for half, pv in (((0, pva), (1, pvb)) if stage in ("pv", "full") else ()):
                        r0 = 64 * half
                        rc = sm.tile([1, TCH], F32, tag="rc")
                        nc.vector.reciprocal(rc[:], pv[64:65, :])
                        bcp = ps_bc.tile([64, TCH], F32, tag="bc")
                        nc.tensor.matmul(bcp[:], ones_r[:], rc[:])
                        bc = sm.tile([64, TCH], F32, tag="bc")
                        nc.vector.tensor_copy(bc[:], bcp[:])
                        nc.vector.tensor_mul(at_c[r0:r0 + 64, m, :], pv[0:64, :], bc[:])
                        nc.vector.tensor_copy(
                            at_h[r0:r0 + 64, m, :], at_c[r0:r0 + 64, m, :])
                        nc.vector.scalar_tensor_tensor(
                            at_l[r0:r0 + 64, m, :], at_c[r0:r0 + 64, m, :], 1.0,
                            at_h[r0:r0 + 64, m, :], ALU.mult, ALU.subtract)

                if stage != "full":
                    continue
                # ---- output projection partial for this chunk (3-pass) ----
                cc_in = dram.tile([C, TCH], F32)
                for mo in range(KT):
                    pso = ps_o.tile([128, TCH], F32)
                    ws = slice(128 * mo, 128 * (mo + 1))
                    for kb in range(KO):
                        first, last = (kb == 0), (kb == KO - 1)
                        nc.tensor.matmul(pso[:], wo_th[:, kb, ws], at_h[:, kb, :],
                                         start=first, stop=False)
                        nc.tensor.matmul(pso[:], wo_th[:, kb, ws], at_l[:, kb, :],
                                         start=False, stop=False)
                        nc.tensor.matmul(pso[:], wo_tl[:, kb, ws], at_h[:, kb, :],
                                         start=False, stop=last)
                    ob = outp.tile([128, TCH], F32)
                    nc.vector.tensor_scalar_add(ob[:], pso[:], b_sb[:, mo:mo + 1])
                    nc.sync.dma_start(cc_in[128 * mo:128 * (mo + 1), :], ob[:])

                cc_out = dram.tile([CL, TCH], F32)
                if collective:
                    nc.gpsimd.collective_compute(
                        "ReduceScatter", mybir.AluOpType.add,
                        replica_groups=[[0, 1], [2, 3], [4, 5], [6, 7]],
                        ins=[cc_in.opt()], outs=[cc_out.opt()])
                else:
                    nc.sync.dma_start(cc_out[:], cc_in[0:CL, :])
                nc.sync.dma_start(out_half[:, ts], cc_out[:])

            if stage != "full":
                nc.sync.dma_start(out_half[0:128, 0:KT], b_sb[:])

    nc.compile()
    return nc


_NC_CACHE = {}


def get_nc(collective=True, reps=1):
    key = (collective, reps)
    if key not in _NC_CACHE:
        _NC_CACHE[key] = build(collective, reps)
    return _NC_CACHE[key]


def _split16(a):
    h = a.astype(np.float16)
    l = (a - h.astype(np.float32)).astype(np.float16)
    return h, l


def make_in_maps(x, w_qkv, w_out, b_out):
    x = np.asarray(x, dtype=np.float32)
    w_qkv = np.asarray(w_qkv, dtype=np.float32)
    w_out = np.asarray(w_out, dtype=np.float32)
    b_out = np.asarray(b_out, dtype=np.float32)
    in_maps = []
    zeros_b = np.zeros_like(b_out)
    for c in range(NCORES):
        bi, g = c // 2, c % 2
        cols = slice(CL * g, CL * (g + 1))
        w_loc = np.ascontiguousarray(np.concatenate(
            [w_qkv[:, cols], w_qkv[:, C:][:, cols], w_qkv[:, 2 * C:][:, cols]],
            axis=1))
        x_t = np.ascontiguousarray(x[bi].T)
        wo_loc = np.ascontiguousarray(w_out[CL * g:CL * (g + 1), :])
        xh, xl = _split16(x_t)
        wh, wl = _split16(w_loc)
        woh, wol = _split16(wo_loc)
        in_maps.append({
            "x_h": xh, "x_l": xl,
            "w_h": wh, "w_l": wl,
            "wo_h": woh, "wo_l": wol,
            "b_eff": b_out if g == 0 else zeros_b,
        })
    return in_maps


def assemble(results):
    out = np.empty((B, T, C), dtype=np.float32)
    for bi in range(B):
        top = results[2 * bi]["out_half"]       # channels 0:512
        bot = results[2 * bi + 1]["out_half"]   # channels 512:1024
        out[bi] = np.concatenate([top, bot], axis=0).T
    return out


def kernel(x, w_qkv, w_out, b_out):
    nc = get_nc(collective=True)
    in_maps = make_in_maps(x, w_qkv, w_out, b_out)
    res = run_bass_kernel_spmd(nc, in_maps, list(range(NCORES)))
    return assemble(res.results)


if __name__ == "__main__":
    nc = build()
    print("instructions:", len(nc.inst_map))
